# revision 1
# baseline (speedup 1.0000x reference)
"""ContextQueryAttention Trainium2 kernel.

Reference computation (per batch b):
    S = (c@wc)[:,None] + (q@wq)[None,:] + (c*wm) @ q.T        # (Lc, Lq)
    S1 = softmax(S, axis=0)  (over context dim i)
    S2 = softmax(S, axis=1)  (over question dim j)
    A  = S1 @ q
    Bm = (S1 @ S2.T) @ c
    out = [c, A, c*A, c*Bm] @ wr + br

Algebraic restructuring used here:
  * Bm = S1 @ (S2.T @ c)   -- avoids the (Lc,Lc) intermediate entirely.
  * q~ = wm*q + wc (per-feature). Then q~ @ c.T = M^T + u[i] where
    u = c@wc; the v[j] = q@wq term is constant along the i-softmax and
    cancels, so S1^T = softmax_free(q~ @ c.T) directly.
  * c @ q~.T = M + u[i]; u[i] is constant along the j-softmax, and v[j]
    is added via a rank-1 matmul (ones x v) into the PSUM accumulation.
  * exp() without max subtraction (inputs are unit-scale gaussians; S
    stays |S| < ~10, far from fp32 overflow).
  * softmax normalizers are folded into downstream operands instead of
    rescaling the big exp(S^T) matrix: A = (q/s1) @ E1, Bm uses
    Y = (S2^T c)/s1.
  * The (Lc,4D)@(4D,D) output projection is done blockwise from the
    d-major (transposed) layouts produced naturally by the PE.
  * Matmuls run in float32r (e8m11, full PE rate) -- all PE operands are
    float32r-typed tiles so producers emit rounded values.

Sharding: pure data parallel over batch: 16 batches -> 8 cores x 2.
"""

import numpy as np

import concourse.bass as bass
import concourse.tile as tile
from concourse import bacc, mybir
from concourse import bass2jax
from concourse.masks import make_identity

N_CORES = 8
B, Lc, Lq, D = 16, 2048, 512, 512
BPC = B // N_CORES  # batches per core

F32 = mybir.dt.float32
F32R = mybir.dt.float32r

AF = mybir.ActivationFunctionType
ALU = mybir.AluOpType
AX = mybir.AxisListType

NT = Lc // 128   # 16 context row-blocks
NG = Lq // 128   # 4 question row-blocks
NK = D // 128    # 4 feature blocks
NC_ = Lc // 512  # 4 i-chunks of 512


def build_program(hybrid=True, repeat=1):
    """hybrid=True: S-score matmuls in float32r (accurate softmax), the
    probability-weighted matmuls (Y/AT/BmT/final) in bf16 (2x PE rate).
    hybrid=False: everything float32r."""
    MD = F32R                      # dtype of score-path PE tiles
    BD = mybir.dt.bfloat16 if hybrid else F32R   # dtype of bulk-path PE tiles

    nc = bacc.Bacc(None, target_bir_lowering=False)

    c2 = nc.declare_dram_parameter("c2", [BPC, Lc, D], F32, isOutput=False)
    q2 = nc.declare_dram_parameter("q2", [BPC, Lq, D], F32, isOutput=False)
    w0 = nc.declare_dram_parameter("w0", [3 * D], F32, isOutput=False)
    wr = nc.declare_dram_parameter("wr", [4 * D, D], F32, isOutput=False)
    br = nc.declare_dram_parameter("br", [D], F32, isOutput=False)
    out2 = nc.declare_dram_parameter("out2", [BPC, Lc, D], F32, isOutput=True)
    ev_dram = nc.dram_tensor("ev_scratch", [Lq], F32)
    s2_dram = nc.dram_tensor("s2_scratch", [Lc], F32)

    # gpsimd (SWDGE) DMAs cast f32 -> f32r / bf16 on the fly.
    def load(out, in_):
        nc.gpsimd.dma_start(out=out, in_=in_)

    with tile.TileContext(nc) as tc:
        with (
            tc.tile_pool(name="sb", bufs=1) as sb,
            tc.tile_pool(name="ps", bufs=2, space="PSUM") as ps,
            tc.tile_pool(name="pt", bufs=2, space="PSUM") as pt,
        ):
            # ---- constants ----
            ident_f = sb.tile([128, 128], F32, tag="identf")
            make_identity(nc, ident_f)
            identb = sb.tile([128, 128], mybir.dt.bfloat16, tag="identb")
            nc.vector.tensor_copy(identb, ident_f)
            ones1_f = sb.tile([1, 128], F32, tag="ones1f")
            nc.vector.memset(ones1_f, 1.0)
            ones1 = sb.tile([1, 128], MD, tag="ones1")
            nc.vector.tensor_copy(ones1, ones1_f)
            ones1b = sb.tile([1, 128], BD, tag="ones1b")
            nc.vector.tensor_copy(ones1b, ones1_f)

            wc_sb = sb.tile([128, NK], F32, tag="wc")
            wm_sb = sb.tile([128, NK], F32, tag="wm")
            wq_sb = sb.tile([128, NK], MD, tag="wq")
            nc.sync.dma_start(out=wc_sb, in_=w0[0:D].rearrange("(k p) -> p k", p=128))
            nc.sync.dma_start(out=wm_sb, in_=w0[2 * D:3 * D].rearrange("(k p) -> p k", p=128))
            load(wq_sb, w0[D:2 * D].rearrange("(k p) -> p k", p=128))

            br_sb = sb.tile([1, D], BD, tag="br")
            load(br_sb, br.rearrange("(a e) -> a e", a=1))

            W_sb = sb.tile([128, 4 * NK, D], BD, tag="W")
            wr_r = wr.rearrange("(t p) e -> p t e", p=128)
            for tq in range(4):
                load(W_sb[:, tq * NK:(tq + 1) * NK, :],
                     wr_r[:, tq * NK:(tq + 1) * NK, :])

            def one_batch(b):
                # ---- load (raw f32 via fast HWDGE) ----
                cSt = sb.tile([128, NT, D], F32, tag="t32a")
                c_r = c2[b].rearrange("(t p) d -> p t d", p=128)
                for tq in range(4):
                    nc.sync.dma_start(out=cSt[:, tq * 4:(tq + 1) * 4, :],
                                      in_=c_r[:, tq * 4:(tq + 1) * 4, :])
                cNb = sb.tile([128, NT, D], BD, tag="t16a")
                for tq in range(4):
                    load(cNb[:, tq * 4:(tq + 1) * 4, :],
                         c_r[:, tq * 4:(tq + 1) * 4, :])
                qSt = sb.tile([128, NG, D], F32, tag="t8n")
                nc.sync.dma_start(out=qSt, in_=q2[b].rearrange("(g p) d -> p g d", p=128))

                # ---- transposes (fp32 PE transpose-mode, 1024-wide groups) ----
                qT = sb.tile([128, NK, Lq], MD, tag="t8q")
                for kd in range(NK):
                    ptile = pt.tile([128, 1024], F32, tag="tr")
                    for g in range(NG):
                        nc.tensor.transpose(
                            ptile[:, g * 128:(g + 1) * 128],
                            qSt[:, g, kd * 128:(kd + 1) * 128], ident_f)
                    nc.any.tensor_copy(qT[:, kd, :], ptile[:, 0:512])

                cT = sb.tile([128, NK, Lc], MD, tag="t32c")
                cTb = sb.tile([128, NK, Lc], BD, tag="t16t")
                for kd in range(NK):
                    for ic2 in range(2):
                        ptile = pt.tile([128, 1024], F32, tag="tr")
                        for t8 in range(8):
                            t = ic2 * 8 + t8
                            nc.tensor.transpose(
                                ptile[:, t8 * 128:(t8 + 1) * 128],
                                cSt[:, t, kd * 128:(kd + 1) * 128], ident_f)
                        sl = slice(ic2 * 1024, (ic2 + 1) * 1024)
                        nc.any.tensor_copy(cT[:, kd, sl], ptile)
                        nc.any.tensor_copy(cTb[:, kd, sl], ptile)

                # ---- v = q @ wq; ev = exp(v) row + column form ----
                pv = ps.tile([128, 1024], F32, tag="mw")
                for kd in range(NK):
                    nc.tensor.matmul(pv[0:1, 0:512], wq_sb[:, kd:kd + 1], qT[:, kd, :],
                                     start=(kd == 0), stop=(kd == NK - 1))
                ev_row = sb.tile([1, Lq], F32, tag="evrow")
                nc.scalar.activation(out=ev_row, in_=pv[0:1, 0:512], func=AF.Exp)
                nc.sync.dma_start(out=ev_dram[:], in_=ev_row[0:1, :])
                ev_colf = sb.tile([128, NG], F32, tag="evcolf")
                nc.sync.dma_start(
                    out=ev_colf, in_=ev_dram.rearrange("(g p) -> p g", p=128))
                ev_colb = sb.tile([128, NG], BD, tag="evcolb")
                nc.vector.tensor_copy(ev_colb, ev_colf)

                # ---- q~T = wm * qT + wc (in place) ----
                for kd in range(NK):
                    nc.vector.tensor_scalar(
                        out=qT[:, kd, :], in0=qT[:, kd, :],
                        scalar1=wm_sb[:, kd:kd + 1], scalar2=wc_sb[:, kd:kd + 1],
                        op0=ALU.mult, op1=ALU.add)

                # ---- ST = q~T.T @ cT -> E1T = exp(ST); s1 row sums ----
                E1T = sb.tile([128, NG, Lc], BD, tag="t16d")
                s1p = sb.tile([128, NG, 2], F32, tag="s1p")
                s1s = sb.tile([128, NG], F32, tag="s1s")
                invs1 = sb.tile([128, NG], F32, tag="invs1")
                for g in range(NG):
                    for ic2 in range(2):
                        pm = ps.tile([128, 1024], F32, tag="mw")
                        for half in range(2):
                            ic = ic2 * 2 + half
                            for kd in range(NK):
                                nc.tensor.matmul(
                                    pm[:, half * 512:(half + 1) * 512],
                                    qT[:, kd, g * 128:(g + 1) * 128],
                                    cT[:, kd, ic * 512:(ic + 1) * 512],
                                    start=(kd == 0), stop=(kd == NK - 1))
                        nc.scalar.activation(
                            out=E1T[:, g, ic2 * 1024:(ic2 + 1) * 1024], in_=pm,
                            func=AF.Exp, accum_out=s1p[:, g, ic2:ic2 + 1])
                    nc.vector.reduce_sum(out=s1s[:, g:g + 1], in_=s1p[:, g, :], axis=AX.X)
                    nc.vector.reciprocal(out=invs1[:, g:g + 1], in_=s1s[:, g:g + 1])

                # ---- s2[i] = sum_j ev[j] * E1T[j,i] ----
                s2row = sb.tile([1, Lc], F32, tag="s2row")
                for ic2 in range(2):
                    ps2 = ps.tile([128, 1024], F32, tag="mw")
                    for half in range(2):
                        ic = ic2 * 2 + half
                        for g in range(NG):
                            nc.tensor.matmul(
                                ps2[0:1, half * 512:(half + 1) * 512],
                                ev_colb[:, g:g + 1],
                                E1T[:, g, ic * 512:(ic + 1) * 512],
                                start=(g == 0), stop=(g == NG - 1))
                    nc.vector.reciprocal(out=s2row[0:1, ic2 * 1024:(ic2 + 1) * 1024],
                                         in_=ps2[0:1, :])
                invs2c = sb.tile([128, NT], F32, tag="invs2c")
                nc.sync.dma_start(out=s2_dram[:], in_=s2row[0:1, :])
                nc.sync.dma_start(
                    out=invs2c, in_=s2_dram.rearrange("(t p) -> p t", p=128))

                # ---- G[i,j] = E1T[j,i] / s2[i]  (bf16 PE transposes) ----
                G = sb.tile([128, NT, Lq], BD, tag="t16b")
                for t in range(NT):
                    ptb = pt.tile([128, 512], BD, tag="tr")
                    for g in range(NG):
                        nc.tensor.transpose(
                            ptb[:, g * 128:(g + 1) * 128],
                            E1T[:, g, t * 128:(t + 1) * 128], identb)
                    nc.vector.tensor_scalar_mul(G[:, t, :], ptb, invs2c[:, t:t + 1])

                # ---- qNb = (q/s1) in bf16, straight from f32 staging ----
                qNb = sb.tile([128, NG, D], BD, tag="t4q")
                for g in range(NG):
                    nc.vector.tensor_scalar_mul(qNb[:, g, :], qSt[:, g, :], invs1[:, g:g + 1])

                # ---- Y = diag(ev/s1) (G.T @ c) ----
                ysc = sb.tile([128, NG], F32, tag="ysc")
                nc.vector.tensor_mul(ysc, ev_colf, invs1)
                Y = sb.tile([128, NG, D], BD, tag="t8n")
                for g2 in range(2):
                    pm = ps.tile([128, 1024], F32, tag="mw")
                    for half in range(2):
                        g = g2 * 2 + half
                        for t in range(NT):
                            nc.tensor.matmul(
                                pm[:, half * 512:(half + 1) * 512],
                                G[:, t, g * 128:(g + 1) * 128], cNb[:, t, :],
                                start=(t == 0), stop=(t == NT - 1))
                    for half in range(2):
                        g = g2 * 2 + half
                        nc.vector.tensor_scalar_mul(
                            Y[:, g, :], pm[:, half * 512:(half + 1) * 512],
                            ysc[:, g:g + 1])

                # ---- AT = (q/s1).T @ E1T ; cAT = cTb*AT, 1024-wide ----
                AT = sb.tile([128, NK, Lc], BD, tag="t16a")
                cAT = sb.tile([128, NK, Lc], BD, tag="t16e")
                for kd in range(NK):
                    for ic2 in range(2):
                        pm = ps.tile([128, 1024], F32, tag="mw")
                        for half in range(2):
                            ic = ic2 * 2 + half
                            for g in range(NG):
                                nc.tensor.matmul(
                                    pm[:, half * 512:(half + 1) * 512],
                                    qNb[:, g, kd * 128:(kd + 1) * 128],
                                    E1T[:, g, ic * 512:(ic + 1) * 512],
                                    start=(g == 0), stop=(g == NG - 1))
                        sl = slice(ic2 * 1024, (ic2 + 1) * 1024)
                        nc.any.tensor_copy(AT[:, kd, sl], pm)
                        nc.vector.tensor_mul(cAT[:, kd, sl], cTb[:, kd, sl], AT[:, kd, sl])

                # ---- BmT = Y.T @ E1T ; cBT in place, 1024-wide ----
                BmT = sb.tile([128, NK, Lc], BD, tag="t16b")
                for kd in range(NK):
                    for ic2 in range(2):
                        pm = ps.tile([128, 1024], F32, tag="mw")
                        for half in range(2):
                            ic = ic2 * 2 + half
                            for g in range(NG):
                                nc.tensor.matmul(
                                    pm[:, half * 512:(half + 1) * 512],
                                    Y[:, g, kd * 128:(kd + 1) * 128],
                                    E1T[:, g, ic * 512:(ic + 1) * 512],
                                    start=(g == 0), stop=(g == NG - 1))
                        sl = slice(ic2 * 1024, (ic2 + 1) * 1024)
                        nc.any.tensor_copy(BmT[:, kd, sl], pm)
                        nc.vector.tensor_mul(BmT[:, kd, sl], BmT[:, kd, sl], cTb[:, kd, sl])

                # ---- out = c@W1 + A@W2 + cA@W3 + cB@W4 + br, paired tiles ----
                for t2 in range(NT // 2):
                    pm = ps.tile([128, 1024], F32, tag="mw")
                    for half in range(2):
                        t = t2 * 2 + half
                        first = True
                        for si, src in enumerate((cTb, AT, cAT, BmT)):
                            for kd in range(NK):
                                nc.tensor.matmul(
                                    pm[:, half * 512:(half + 1) * 512],
                                    src[:, kd, t * 128:(t + 1) * 128],
                                    W_sb[:, si * NK + kd, :], start=first, stop=False)
                                first = False
                        nc.tensor.matmul(pm[:, half * 512:(half + 1) * 512],
                                         ones1b, br_sb, start=False, stop=True)
                    ot = sb.tile([128, 2, 512], F32, tag="outst", bufs=3)
                    nc.any.tensor_copy(ot, pm)
                    nc.sync.dma_start(
                        out=out2[b].rearrange("(u p) e -> p u e", p=128)[:, t2 * 2:t2 * 2 + 2, :],
                        in_=ot)

            if repeat > 1:
                # timing harness only: repeat the whole workload on-device so
                # per-call dispatch overhead can be subtracted out
                hints = (mybir.EngineType.PE, mybir.EngineType.DVE,
                         mybir.EngineType.Activation, mybir.EngineType.SP,
                         mybir.EngineType.Pool)
                with tc.For_i(0, repeat, 1, hint_engines=hints):
                    for b in range(BPC):
                        one_batch(b)
            else:
                for b in range(BPC):
                    one_batch(b)

    nc.compile()
    return nc


class Runner:
    """Persistent SPMD runner: jit once, execute many times.

    Mirrors concourse.bass2jax.run_bass_via_pjrt's multi-core path but keeps
    the compiled executable so repeated calls don't recompile.
    """

    def __init__(self, nc):
        import jax
        from jax.experimental.shard_map import shard_map
        from jax.sharding import Mesh, PartitionSpec

        bass2jax.install_neuronx_cc_hook()
        self.nc = nc
        self.jax = jax

        partition_name = (
            nc.partition_id_tensor.name if nc.partition_id_tensor else None
        )
        in_names, out_names, out_avals, zero_shapes = [], [], [], []
        for alloc in nc.m.functions[0].allocations:
            if not isinstance(alloc, mybir.MemoryLocationSet):
                continue
            name = alloc.memorylocations[0].name
            if alloc.kind == "ExternalInput":
                if name != partition_name:
                    in_names.append(name)
            elif alloc.kind == "ExternalOutput":
                shape = tuple(alloc.tensor_shape)
                dtype = mybir.dt.np(alloc.dtype)
                out_names.append(name)
                out_avals.append(jax.core.ShapedArray(shape, dtype))
                zero_shapes.append((shape, dtype))
        self.in_names = list(in_names)
        self.out_names = out_names
        self.out_avals = out_avals
        self.zero_shapes = zero_shapes
        n_params = len(in_names)
        n_outs = len(out_names)

        all_in_names = list(in_names) + list(out_names)
        if partition_name is not None:
            all_in_names.append(partition_name)

        def _body(*args):
            operands = list(args)
            if partition_name is not None:
                operands.append(bass2jax.partition_id_tensor())
            outs = bass2jax._bass_exec_p.bind(
                *operands,
                out_avals=tuple(out_avals),
                in_names=tuple(all_in_names),
                out_names=tuple(out_names),
                lowering_input_output_aliases=(),
                sim_require_finite=True,
                sim_require_nnan=True,
                nc=nc,
            )
            return tuple(outs)

        devices = jax.devices()[:N_CORES]
        mesh = Mesh(np.asarray(devices), ("core",))
        in_specs = (PartitionSpec("core"),) * (n_params + n_outs)
        out_specs = (PartitionSpec("core"),) * n_outs
        self.fn = jax.jit(
            shard_map(_body, mesh=mesh, in_specs=in_specs,
                      out_specs=out_specs, check_rep=False),
            keep_unused=True,
        )

    def concat_inputs(self, in_maps):
        return [
            np.concatenate([np.asarray(m[name]) for m in in_maps], axis=0)
            for name in self.in_names
        ]

    def zeros(self):
        return [
            np.zeros((N_CORES * s[0], *s[1:]), d) for (s, d) in self.zero_shapes
        ]

    def run_device(self, concat_in, zeros):
        """Execute; returns list of global (N_CORES*dim0, ...) jax arrays."""
        out = self.fn(*concat_in, *zeros)
        self.jax.block_until_ready(out)
        return out

    def run(self, in_maps):
        outs = self.run_device(self.concat_inputs(in_maps), self.zeros())
        return [
            {
                name: np.asarray(outs[i]).reshape(
                    N_CORES, *self.out_avals[i].shape)[c]
                for i, name in enumerate(self.out_names)
            }
            for c in range(N_CORES)
        ]


_CACHED = {}


def _get_runner(**kw):
    key = tuple(sorted(kw.items()))
    if key not in _CACHED:
        _CACHED[key] = Runner(build_program(**kw))
    return _CACHED[key]


def make_in_maps(context, question, w0, wr, br):
    return [
        {
            "c2": context[c * BPC:(c + 1) * BPC],
            "q2": question[c * BPC:(c + 1) * BPC],
            "w0": w0,
            "wr": wr,
            "br": br,
        }
        for c in range(N_CORES)
    ]


def kernel(context, question, w0, wr, br):
    context = np.ascontiguousarray(np.asarray(context, dtype=np.float32))
    question = np.ascontiguousarray(np.asarray(question, dtype=np.float32))
    w0 = np.ascontiguousarray(np.asarray(w0, dtype=np.float32))
    wr = np.ascontiguousarray(np.asarray(wr, dtype=np.float32))
    br = np.ascontiguousarray(np.asarray(br, dtype=np.float32))

    runner = _get_runner()
    res = runner.run(make_in_maps(context, question, w0, wr, br))
    return np.concatenate([res[c]["out2"] for c in range(N_CORES)], axis=0)



# revision 13
# speedup vs baseline: 1.0806x; 1.0806x over previous
"""ContextQueryAttention Trainium2 kernel.

Reference computation (per batch b):
    S = (c@wc)[:,None] + (q@wq)[None,:] + (c*wm) @ q.T        # (Lc, Lq)
    S1 = softmax(S, axis=0)  (over context dim i)
    S2 = softmax(S, axis=1)  (over question dim j)
    A  = S1 @ q
    Bm = (S1 @ S2.T) @ c
    out = [c, A, c*A, c*Bm] @ wr + br

Algebraic restructuring (same as the bf16 baseline):
  * Bm = S1 @ (S2.T @ c)   -- avoids the (Lc,Lc) intermediate entirely.
  * q~ = wm*q + wc (per-feature). Then ST = q~ @ c.T gives the i-softmax
    logits directly (the q@wq term is constant along i and cancels).
  * exp() without max subtraction (S is unit-scale; E = e^S <= ~240,
    which also fits fp8e4m3's range).
  * softmax normalizers fold into downstream operands, never rescaling
    the big exp(ST) matrix.

Precision plan (validated in numpy against the fp32 reference):
  * score path f32r (full-rate on PE for wide moving operands).
  * E1T, ev, G, q/s1, Y, c stored fp8e4m3; the four probability-weighted
    matmuls (s2, AT, Y, BmT) run in DoubleRow perf mode (2 k-tiles per
    instruction, 0.5 cyc/row).
  * q/s1 and Y are pre-scaled by QS/YS (pow2) before fp8 quantization to
    stay in e4m3's normal range; descale folds into PSUM->SBUF copies.
  * final (Lc,4D)@(4D,D) projection stays bf16 (fp8 fails tolerance).

Sharding: pure data parallel over batch: 16 batches -> 8 cores x 2.
"""

import numpy as np

import concourse.bass as bass
import concourse.tile as tile
from concourse import bacc, mybir
from concourse import bass2jax
from concourse.masks import make_identity

N_CORES = 8
B, Lc, Lq, D = 16, 2048, 512, 512
BPC = B // N_CORES  # batches per core

F32 = mybir.dt.float32
F32R = mybir.dt.float32r
BF16 = mybir.dt.bfloat16
FP8 = mybir.dt.float8e4

AF = mybir.ActivationFunctionType
ALU = mybir.AluOpType
AX = mybir.AxisListType
DR = mybir.MatmulPerfMode.DoubleRow

NT = Lc // 128   # 16 context row-blocks
NG = Lq // 128   # 4 question row-blocks
NK = D // 128    # 4 feature blocks

QS = 16384.0     # fp8 pre-scale for q/s1
YS = 8192.0      # fp8 pre-scale for Y


def build_program():
    nc = bacc.Bacc(None, target_bir_lowering=False)

    # f32r tiles are bit-compatible with f32; np mapping stays float32.
    c2 = nc.declare_dram_parameter("c2", [BPC, Lc, D], F32R, isOutput=False)
    q2 = nc.declare_dram_parameter("q2", [BPC, Lq, D], F32R, isOutput=False)
    w0 = nc.declare_dram_parameter("w0", [3 * D], F32, isOutput=False)
    wr = nc.declare_dram_parameter("wr", [4 * D, D], F32, isOutput=False)
    br = nc.declare_dram_parameter("br", [D], F32, isOutput=False)
    out2 = nc.declare_dram_parameter("out2", [BPC, Lc, D], F32, isOutput=True)
    ev_dram = nc.dram_tensor("ev_scratch", [Lq], F32)
    s2_dram = nc.dram_tensor("s2_scratch", [Lc], F32)

    with tile.TileContext(nc) as tc:
        with (
            tc.tile_pool(name="sb", bufs=1) as sb,
            tc.tile_pool(name="ps", bufs=2, space="PSUM") as ps,
            tc.tile_pool(name="pt", bufs=2, space="PSUM") as pt,
        ):
            # ---- constants ----
            ident_f = sb.tile([128, 128], F32, tag="identf")
            make_identity(nc, ident_f)
            ident_r = sb.tile([128, 128], F32R, tag="identr")
            nc.vector.tensor_copy(ident_r, ident_f)
            ident8 = sb.tile([128, 128], FP8, tag="ident8")
            nc.vector.tensor_copy(ident8, ident_f)
            ones1_f = sb.tile([1, 128], F32, tag="ones1f")
            nc.vector.memset(ones1_f, 1.0)
            ones1b = sb.tile([1, 128], BF16, tag="ones1b")
            nc.vector.tensor_copy(ones1b, ones1_f)

            wc_sb = sb.tile([128, NK], F32, tag="wc")
            wm_sb = sb.tile([128, NK], F32, tag="wm")
            wq_sb = sb.tile([128, NK], F32R, tag="wq")
            nc.sync.dma_start(out=wc_sb, in_=w0[0:D].rearrange("(k p) -> p k", p=128))
            nc.sync.dma_start(out=wm_sb, in_=w0[2 * D:3 * D].rearrange("(k p) -> p k", p=128))
            nc.gpsimd.dma_start(out=wq_sb, in_=w0[D:2 * D].rearrange("(k p) -> p k", p=128))

            br_sb = sb.tile([1, D], BF16, tag="br")
            nc.gpsimd.dma_start(out=br_sb, in_=br.rearrange("(a e) -> a e", a=1))

            # ---- batch input staging loads (batch 0 first, then
            #      prefetched from inside one_batch for b+1) ----
            wr_r = wr.rearrange("(t p) e -> p t e", p=128)

            def load_stage(b):
                c_rb = c2[b].rearrange("(t p) d -> p t d", p=128)
                cSt_b = sb.tile([128, NT, D], F32R, tag="cstage")
                for tq in range(4):
                    nc.sync.dma_start(out=cSt_b[:, tq * 4:(tq + 1) * 4, :],
                                      in_=c_rb[:, tq * 4:(tq + 1) * 4, :])
                qSt_b = sb.tile([128, NG, D], F32R, tag="qstage")
                nc.sync.dma_start(
                    out=qSt_b, in_=q2[b].rearrange("(g p) d -> p g d", p=128))
                return cSt_b, qSt_b

            stage0 = load_stage(0)

            # ---- wr: f32 HWDGE staging + Pool cast to bf16 (eighths) ----
            W_sb = sb.tile([128, 4 * NK, D], BF16, tag="W")
            for eighth in range(8):
                Wst = sb.tile([128, 2, D], F32, tag="Wst", bufs=2)
                nc.sync.dma_start(out=Wst, in_=wr_r[:, eighth * 2:(eighth + 1) * 2, :])
                nc.gpsimd.tensor_copy(W_sb[:, eighth * 2:(eighth + 1) * 2, :], Wst)

            def one_batch(b, stage):
                cSt_b, qSt_b = stage
                next_stage = None

                # ---- transposes (f32r PE transpose, 1.5 cyc/row) ----
                qT = sb.tile([128, NK, Lq], F32R, tag="qT")
                for kd in range(NK):
                    ptile = pt.tile([128, 1024], F32R, tag="tr")
                    for g in range(NG):
                        nc.tensor.transpose(
                            ptile[:, g * 128:(g + 1) * 128],
                            qSt_b[:, g, kd * 128:(kd + 1) * 128], ident_r)
                    nc.vector.tensor_copy(qT[:, kd, :], ptile[:, 0:512])

                # ---- cN8: fp8 copy of c in natural layout (Pool engine) ----
                cN8 = sb.tile([128, NT, D], FP8, tag="cN8")
                for tq in range(4):
                    nc.gpsimd.tensor_copy(cN8[:, tq * 4:(tq + 1) * 4, :],
                                          cSt_b[:, tq * 4:(tq + 1) * 4, :])

                # ---- v = q @ wq; ev = exp(v) row + column forms ----
                pv = ps.tile([128, 1024], F32, tag="mw")
                for kd in range(NK):
                    nc.tensor.matmul(pv[0:1, 0:512], wq_sb[:, kd:kd + 1], qT[:, kd, :],
                                     start=(kd == 0), stop=(kd == NK - 1))
                ev_row = sb.tile([1, Lq], F32, tag="evrow")
                nc.scalar.activation(out=ev_row, in_=pv[0:1, 0:512], func=AF.Exp)
                nc.sync.dma_start(out=ev_dram[:], in_=ev_row[0:1, :])
                ev_colf = sb.tile([128, NG], F32, tag="evcolf")
                nc.sync.dma_start(
                    out=ev_colf, in_=ev_dram.rearrange("(g p) -> p g", p=128))
                ev_col8 = sb.tile([128, NG, 1], FP8, tag="evcol8")
                nc.vector.tensor_copy(ev_col8[:, :, 0], ev_colf)

                # ---- q~T = wm * qT + wc (in place) ----
                for kd in range(NK):
                    nc.vector.tensor_scalar(
                        out=qT[:, kd, :], in0=qT[:, kd, :],
                        scalar1=wm_sb[:, kd:kd + 1], scalar2=wc_sb[:, kd:kd + 1],
                        op0=ALU.mult, op1=ALU.add)

                # ---- chunked c transposes + ST = q~T.T @ cT -> E1T8 ----
                # cT lives as a rotating [128, NK, 512] chunk; per chunk,
                # transpose 4 context row-blocks then score all 4 g-blocks.
                cTb = sb.tile([128, NK, Lc], BF16, tag="cTb")
                E1T8 = sb.tile([128, NG, Lc], FP8, tag="E8")
                s1p = sb.tile([128, NG, 4], F32, tag="s1p")
                s1s = sb.tile([128, NG], F32, tag="s1s")
                invs1 = sb.tile([128, NG], F32, tag="invs1")
                invs1q = sb.tile([128, NG], F32, tag="invs1q")
                qn8 = sb.tile([128, NG, D], FP8, tag="qn8")
                for ic in range(4):
                    cTc = sb.tile([128, NK, 512], F32R, tag="cTc", bufs=2)
                    sl = slice(ic * 512, (ic + 1) * 512)
                    for kd in range(NK):
                        ptile = pt.tile([128, 512], F32R, tag="tr")
                        for t4 in range(4):
                            t = ic * 4 + t4
                            nc.tensor.transpose(
                                ptile[:, t4 * 128:(t4 + 1) * 128],
                                cSt_b[:, t, kd * 128:(kd + 1) * 128], ident_r)
                        nc.vector.tensor_copy(cTc[:, kd, :], ptile)
                        nc.scalar.activation(out=cTb[:, kd, sl], in_=ptile, func=AF.Copy)
                    for g in range(NG):
                        pm = ps.tile([128, 512], F32, tag="mw")
                        for kd in range(NK):
                            nc.tensor.matmul(
                                pm, qT[:, kd, g * 128:(g + 1) * 128],
                                cTc[:, kd, :],
                                start=(kd == 0), stop=(kd == NK - 1))
                        nc.scalar.activation(
                            out=E1T8[:, g, sl], in_=pm,
                            func=AF.Exp, accum_out=s1p[:, g, ic:ic + 1])
                for g in range(NG):
                    nc.vector.reduce_sum(out=s1s[:, g:g + 1], in_=s1p[:, g, :], axis=AX.X)
                    nc.vector.reciprocal(out=invs1[:, g:g + 1], in_=s1s[:, g:g + 1])
                    nc.vector.tensor_scalar_mul(invs1q[:, g:g + 1], invs1[:, g:g + 1], QS)
                    # qn8 = (q/s1)*QS in fp8, from f32r staging
                    nc.vector.tensor_scalar_mul(qn8[:, g, :], qSt_b[:, g, :],
                                                invs1q[:, g:g + 1])

                # ---- s2[i] = sum_j ev[j] * E1T[j,i]  (fp8, plain) ----
                s2row = sb.tile([1, Lc], F32, tag="s2row")
                for ic2 in range(2):
                    ps2 = ps.tile([128, 1024], F32, tag="mw")
                    for half in range(2):
                        ic = ic2 * 2 + half
                        for g in range(NG):
                            nc.tensor.matmul(
                                ps2[0:1, half * 512:(half + 1) * 512],
                                ev_col8[:, g, 0:1],
                                E1T8[:, g, ic * 512:(ic + 1) * 512],
                                start=(g == 0), stop=(g == NG - 1))
                    nc.scalar.activation(out=s2row[0:1, ic2 * 1024:(ic2 + 1) * 1024],
                                         in_=ps2[0:1, :], func=AF.Copy)
                nc.sync.dma_start(out=s2_dram[:], in_=s2row[0:1, :])
                s2c = sb.tile([128, NT], F32, tag="s2c")
                nc.sync.dma_start(out=s2c, in_=s2_dram.rearrange("(t p) -> p t", p=128))
                invs2c = sb.tile([128, NT], F32, tag="invs2c")
                nc.vector.reciprocal(out=invs2c, in_=s2c)

                # ---- AT = (q/s1).T @ E1T fp8-DR; cAT = cTb*AT ----
                # (issued before the G transposes so the s2 DMA roundtrip
                #  hides behind PE work)
                AT = sb.tile([128, NK, Lc], BF16, tag="AT")
                cAT = sb.tile([128, NK, Lc], BF16, tag="cAT")
                for ic2 in range(2):
                    for kd in range(NK):
                        pm = ps.tile([128, 1024], F32, tag="mw")
                        for half in range(2):
                            ic = ic2 * 2 + half
                            for g2 in range(NG // 2):
                                nc.tensor.matmul(
                                    pm[:, half * 512:(half + 1) * 512],
                                    qn8[:, g2 * 2:g2 * 2 + 2, kd * 128:(kd + 1) * 128],
                                    E1T8[:, g2 * 2:g2 * 2 + 2, ic * 512:(ic + 1) * 512],
                                    start=(g2 == 0), stop=(g2 == NG // 2 - 1),
                                    perf_mode=DR)
                        sl = slice(ic2 * 1024, (ic2 + 1) * 1024)
                        nc.scalar.activation(out=AT[:, kd, sl], in_=pm,
                                             func=AF.Copy, scale=1.0 / QS)
                        nc.vector.tensor_mul(cAT[:, kd, sl], cTb[:, kd, sl], AT[:, kd, sl])

                # ---- G[i,j] = E1T[j,i] / s2[i]  (fp8 PE transposes) ----
                # (fp8 PE transpose writes with element step 2; read strided)
                G8 = sb.tile([128, NT, Lq], FP8, tag="G8")
                for t in range(NT):
                    ptb = pt.tile([128, 1024], FP8, tag="tr")
                    for g in range(NG):
                        nc.tensor.transpose(
                            ptb[:, g * 256:(g + 1) * 256:2],
                            E1T8[:, g, t * 128:(t + 1) * 128], ident8)
                    nc.vector.tensor_scalar_mul(G8[:, t, :], ptb[:, 0:1024:2],
                                                invs2c[:, t:t + 1])

                # ---- Y = diag(ev/s1*YS) (G.T @ c)  (fp8 DR) ----
                ysc = sb.tile([128, NG], F32, tag="ysc")
                nc.vector.tensor_mul(ysc, ev_colf, invs1)
                ysc_s = sb.tile([128, NG], F32, tag="yscs")
                nc.vector.tensor_scalar_mul(ysc_s, ysc, YS)
                Y8 = sb.tile([128, NG, D], FP8, tag="Y8")
                for g2 in range(2):
                    pm = ps.tile([128, 1024], F32, tag="mw")
                    for half in range(2):
                        g = g2 * 2 + half
                        for t2 in range(NT // 2):
                            nc.tensor.matmul(
                                pm[:, half * 512:(half + 1) * 512],
                                G8[:, t2 * 2:t2 * 2 + 2, g * 128:(g + 1) * 128],
                                cN8[:, t2 * 2:t2 * 2 + 2, :],
                                start=(t2 == 0), stop=(t2 == NT // 2 - 1),
                                perf_mode=DR)
                    for half in range(2):
                        g = g2 * 2 + half
                        nc.vector.tensor_scalar_mul(
                            Y8[:, g, :], pm[:, half * 512:(half + 1) * 512],
                            ysc_s[:, g:g + 1])

                # ---- BmT = Y.T @ E1T (fp8 DR); times cTb, descale 1/YS ----
                BmT = sb.tile([128, NK, Lc], BF16, tag="BmT")
                for ic2 in range(2):
                    for kd in range(NK):
                        pm = ps.tile([128, 1024], F32, tag="mw")
                        for half in range(2):
                            ic = ic2 * 2 + half
                            for g2 in range(NG // 2):
                                nc.tensor.matmul(
                                    pm[:, half * 512:(half + 1) * 512],
                                    Y8[:, g2 * 2:g2 * 2 + 2, kd * 128:(kd + 1) * 128],
                                    E1T8[:, g2 * 2:g2 * 2 + 2, ic * 512:(ic + 1) * 512],
                                    start=(g2 == 0), stop=(g2 == NG // 2 - 1),
                                    perf_mode=DR)
                        sl = slice(ic2 * 1024, (ic2 + 1) * 1024)
                        nc.scalar.activation(out=BmT[:, kd, sl], in_=pm,
                                             func=AF.Copy, scale=1.0 / YS)
                        nc.vector.tensor_mul(BmT[:, kd, sl], BmT[:, kd, sl], cTb[:, kd, sl])

                # ---- prefetch next batch's inputs (WAR deps via tags) ----
                if b + 1 < BPC:
                    next_stage = load_stage(b + 1)

                # ---- out = c@W1 + A@W2 + cA@W3 + cB@W4 + br ----
                for t2 in range(NT // 2):
                    pm = ps.tile([128, 1024], F32, tag="mw")
                    for half in range(2):
                        t = t2 * 2 + half
                        first = True
                        for si, src in enumerate((cTb, AT, cAT, BmT)):
                            for kd in range(NK):
                                nc.tensor.matmul(
                                    pm[:, half * 512:(half + 1) * 512],
                                    src[:, kd, t * 128:(t + 1) * 128],
                                    W_sb[:, si * NK + kd, :], start=first, stop=False)
                                first = False
                        nc.tensor.matmul(pm[:, half * 512:(half + 1) * 512],
                                         ones1b, br_sb, start=False, stop=True)
                    ot = sb.tile([128, 2, 512], F32, tag="outst", bufs=3)
                    nc.vector.tensor_copy(ot, pm)
                    nc.sync.dma_start(
                        out=out2[b].rearrange("(u p) e -> p u e", p=128)[:, t2 * 2:t2 * 2 + 2, :],
                        in_=ot)
                return next_stage

            stage = stage0
            for b in range(BPC):
                stage = one_batch(b, stage)

    nc.compile()
    return nc


class Runner:
    """Persistent SPMD runner: jit once, execute many times."""

    def __init__(self, nc):
        import jax
        from jax.experimental.shard_map import shard_map
        from jax.sharding import Mesh, PartitionSpec

        bass2jax.install_neuronx_cc_hook()
        self.nc = nc
        self.jax = jax

        partition_name = (
            nc.partition_id_tensor.name if nc.partition_id_tensor else None
        )
        in_names, out_names, out_avals, zero_shapes = [], [], [], []
        for alloc in nc.m.functions[0].allocations:
            if not isinstance(alloc, mybir.MemoryLocationSet):
                continue
            name = alloc.memorylocations[0].name
            if alloc.kind == "ExternalInput":
                if name != partition_name:
                    in_names.append(name)
            elif alloc.kind == "ExternalOutput":
                shape = tuple(alloc.tensor_shape)
                dtype = mybir.dt.np(alloc.dtype)
                out_names.append(name)
                out_avals.append(jax.core.ShapedArray(shape, dtype))
                zero_shapes.append((shape, dtype))
        self.in_names = list(in_names)
        self.out_names = out_names
        self.out_avals = out_avals
        self.zero_shapes = zero_shapes
        n_params = len(in_names)
        n_outs = len(out_names)

        all_in_names = list(in_names) + list(out_names)
        if partition_name is not None:
            all_in_names.append(partition_name)

        def _body(*args):
            operands = list(args)
            if partition_name is not None:
                operands.append(bass2jax.partition_id_tensor())
            outs = bass2jax._bass_exec_p.bind(
                *operands,
                out_avals=tuple(out_avals),
                in_names=tuple(all_in_names),
                out_names=tuple(out_names),
                lowering_input_output_aliases=(),
                sim_require_finite=True,
                sim_require_nnan=True,
                nc=nc,
            )
            return tuple(outs)

        devices = jax.devices()[:N_CORES]
        mesh = Mesh(np.asarray(devices), ("core",))
        in_specs = (PartitionSpec("core"),) * (n_params + n_outs)
        out_specs = (PartitionSpec("core"),) * n_outs
        self.fn = jax.jit(
            shard_map(_body, mesh=mesh, in_specs=in_specs,
                      out_specs=out_specs, check_rep=False),
            keep_unused=True,
        )

    def concat_inputs(self, in_maps):
        return [
            np.concatenate([np.asarray(m[name]) for m in in_maps], axis=0)
            for name in self.in_names
        ]

    def zeros(self):
        return [
            np.zeros((N_CORES * s[0], *s[1:]), d) for (s, d) in self.zero_shapes
        ]

    def run_device(self, concat_in, zeros):
        """Execute; returns list of global (N_CORES*dim0, ...) jax arrays."""
        out = self.fn(*concat_in, *zeros)
        self.jax.block_until_ready(out)
        return out

    def run(self, in_maps):
        outs = self.run_device(self.concat_inputs(in_maps), self.zeros())
        return [
            {
                name: np.asarray(outs[i]).reshape(
                    N_CORES, *self.out_avals[i].shape)[c]
                for i, name in enumerate(self.out_names)
            }
            for c in range(N_CORES)
        ]


_CACHED = {}


def _get_runner(**kw):
    key = tuple(sorted(kw.items()))
    if key not in _CACHED:
        _CACHED[key] = Runner(build_program(**kw))
    return _CACHED[key]


def make_in_maps(context, question, w0, wr, br):
    return [
        {
            "c2": context[c * BPC:(c + 1) * BPC],
            "q2": question[c * BPC:(c + 1) * BPC],
            "w0": w0,
            "wr": wr,
            "br": br,
        }
        for c in range(N_CORES)
    ]


def kernel(context, question, w0, wr, br):
    context = np.ascontiguousarray(np.asarray(context, dtype=np.float32))
    question = np.ascontiguousarray(np.asarray(question, dtype=np.float32))
    w0 = np.ascontiguousarray(np.asarray(w0, dtype=np.float32))
    wr = np.ascontiguousarray(np.asarray(wr, dtype=np.float32))
    br = np.ascontiguousarray(np.asarray(br, dtype=np.float32))

    runner = _get_runner()
    res = runner.run(make_in_maps(context, question, w0, wr, br))
    return np.concatenate([res[c]["out2"] for c in range(N_CORES)], axis=0)


# revision 28
# speedup vs baseline: 1.1012x; 1.0190x over previous
"""ContextQueryAttention Trainium2 kernel.

Reference computation (per batch b):
    S = (c@wc)[:,None] + (q@wq)[None,:] + (c*wm) @ q.T        # (Lc, Lq)
    S1 = softmax(S, axis=0)  (over context dim i)
    S2 = softmax(S, axis=1)  (over question dim j)
    A  = S1 @ q
    Bm = (S1 @ S2.T) @ c
    out = [c, A, c*A, c*Bm] @ wr + br

Algebraic restructuring (same as the bf16 baseline):
  * Bm = S1 @ (S2.T @ c)   -- avoids the (Lc,Lc) intermediate entirely.
  * q~ = wm*q + wc (per-feature). Then ST = q~ @ c.T gives the i-softmax
    logits directly (the q@wq term is constant along i and cancels).
  * exp() without max subtraction (S is unit-scale; E = e^S <= ~240,
    which also fits fp8e4m3's range).
  * softmax normalizers fold into downstream operands, never rescaling
    the big exp(ST) matrix.

Precision plan (validated in numpy against the fp32 reference):
  * score path f32r (full-rate on PE for wide moving operands).
  * E1T, ev, G, q/s1, Y, c stored fp8e4m3; the four probability-weighted
    matmuls (s2, AT, Y, BmT) run in DoubleRow perf mode (2 k-tiles per
    instruction, 0.5 cyc/row).
  * q/s1 and Y are pre-scaled by QS/YS (pow2) before fp8 quantization to
    stay in e4m3's normal range; descale folds into PSUM->SBUF copies.
  * final (Lc,4D)@(4D,D) projection stays bf16 (fp8 fails tolerance).

Sharding: pure data parallel over batch: 16 batches -> 8 cores x 2.
"""

import numpy as np

import concourse.bass as bass
import concourse.tile as tile
from concourse import bacc, mybir
from concourse import bass2jax
from concourse.masks import make_identity

N_CORES = 8
B, Lc, Lq, D = 16, 2048, 512, 512
BPC = B // N_CORES  # batches per core

F32 = mybir.dt.float32
F32R = mybir.dt.float32r
BF16 = mybir.dt.bfloat16
FP8 = mybir.dt.float8e4

AF = mybir.ActivationFunctionType
ALU = mybir.AluOpType
AX = mybir.AxisListType
DR = mybir.MatmulPerfMode.DoubleRow

NT = Lc // 128   # 16 context row-blocks
NG = Lq // 128   # 4 question row-blocks
NK = D // 128    # 4 feature blocks

QS = 16384.0     # fp8 pre-scale for q/s1
YS = 8192.0      # fp8 pre-scale for Y


def build_program():
    nc = bacc.Bacc(None, target_bir_lowering=False)

    # f32r tiles are bit-compatible with f32; np mapping stays float32.
    c2 = nc.declare_dram_parameter("c2", [BPC, Lc, D], F32R, isOutput=False)
    q2 = nc.declare_dram_parameter("q2", [BPC, Lq, D], F32R, isOutput=False)
    w0 = nc.declare_dram_parameter("w0", [3 * D], F32, isOutput=False)
    wr = nc.declare_dram_parameter("wr", [4 * D, D], F32, isOutput=False)
    br = nc.declare_dram_parameter("br", [D], F32, isOutput=False)
    out2 = nc.declare_dram_parameter("out2", [BPC, Lc, D], F32, isOutput=True)
    ev_dram = nc.dram_tensor("ev_scratch", [Lq], F32)
    s2_dram = nc.dram_tensor("s2_scratch", [Lc], BF16)

    with tile.TileContext(nc) as tc:
        with (
            tc.tile_pool(name="sb", bufs=1) as sb,
            tc.tile_pool(name="ps", bufs=2, space="PSUM") as ps,
            tc.tile_pool(name="pt", bufs=2, space="PSUM") as pt,
        ):
            # ---- constants ----
            ident_f = sb.tile([128, 128], F32, tag="identf")
            make_identity(nc, ident_f)
            ident_r = sb.tile([128, 128], F32R, tag="identr")
            nc.vector.tensor_copy(ident_r, ident_f)
            ident8 = sb.tile([128, 128], FP8, tag="ident8")
            nc.vector.tensor_copy(ident8, ident_f)
            ones1_f = sb.tile([1, 128], F32, tag="ones1f")
            nc.vector.memset(ones1_f, 1.0)
            ones1b = sb.tile([1, 128], BF16, tag="ones1b")
            nc.vector.tensor_copy(ones1b, ones1_f)

            wc_sb = sb.tile([128, NK], F32, tag="wc")
            wm_sb = sb.tile([128, NK], F32, tag="wm")
            wq_sb = sb.tile([128, NK], F32R, tag="wq")
            nc.sync.dma_start(out=wc_sb, in_=w0[0:D].rearrange("(k p) -> p k", p=128))
            nc.sync.dma_start(out=wm_sb, in_=w0[2 * D:3 * D].rearrange("(k p) -> p k", p=128))
            nc.gpsimd.dma_start(out=wq_sb, in_=w0[D:2 * D].rearrange("(k p) -> p k", p=128))

            # wcm = wc/wm, folded into the q transpose copy; the score matmul
            # becomes (q + wcm)^T (wm*c) and ev gets an exp-bias correction
            # K = sum_d wq_d*wcm_d (error from this rewrite is O(wc*2^-11)).
            wcm_sb = sb.tile([128, NK], F32, tag="wcm")
            nc.vector.reciprocal(out=wcm_sb, in_=wm_sb)
            nc.vector.tensor_mul(wcm_sb, wcm_sb, wc_sb)
            wqm = sb.tile([128, NK], F32, tag="wqm")
            nc.vector.tensor_mul(wqm, wq_sb, wcm_sb)
            ones_col = sb.tile([128, 1], F32, tag="onescol")
            nc.vector.memset(ones_col, 1.0)
            pk = ps.tile([1, 8], F32, tag="mw")
            nc.tensor.matmul(pk[0:1, 0:NK], ones_col, wqm, start=True, stop=True)
            negK = sb.tile([1, 1], F32, tag="negK")
            nc.vector.reduce_sum(out=negK, in_=pk[0:1, 0:NK], axis=AX.X)
            nc.vector.tensor_scalar_mul(negK, negK, -1.0)

            br_sb = sb.tile([1, D], BF16, tag="br")
            nc.gpsimd.dma_start(out=br_sb, in_=br.rearrange("(a e) -> a e", a=1))

            # ---- batch input staging loads (batch 0 first, then
            #      prefetched from inside one_batch for b+1) ----
            wr_r = wr.rearrange("(t p) e -> p t e", p=128)

            def load_stage(b):
                # q first: the PE pipeline starts with the q transposes
                qSt_b = sb.tile([128, NG, D], F32R, tag="qstage")
                nc.sync.dma_start(
                    out=qSt_b, in_=q2[b].rearrange("(g p) d -> p g d", p=128))
                c_rb = c2[b].rearrange("(t p) d -> p t d", p=128)
                cSt_b = sb.tile([128, NT, D], F32R, tag="cstage")
                for tq in range(4):
                    nc.sync.dma_start(out=cSt_b[:, tq * 4:(tq + 1) * 4, :],
                                      in_=c_rb[:, tq * 4:(tq + 1) * 4, :])
                return cSt_b, qSt_b

            stage0 = load_stage(0)

            # ---- wr: f32 HWDGE staging + Pool cast to bf16 (16ths) ----
            W_sb = sb.tile([128, 4 * NK, D], BF16, tag="W")
            for part in range(16):
                Wst = sb.tile([128, 1, D], F32, tag="Wst", bufs=2)
                nc.sync.dma_start(out=Wst, in_=wr_r[:, part:part + 1, :])
                nc.gpsimd.tensor_copy(W_sb[:, part:part + 1, :], Wst)

            def one_batch(b, stage):
                cSt_b, qSt_b = stage
                next_stage = None

                # ---- q transposes; qTp = qT + wc/wm fused on the copy ----
                qTp = sb.tile([128, NK, Lq], F32R, tag="qTp")
                for kd in range(NK):
                    ptile = pt.tile([128, 1024], F32R, tag="tr")
                    for g in range(NG):
                        nc.tensor.transpose(
                            ptile[:, g * 128:(g + 1) * 128],
                            qSt_b[:, g, kd * 128:(kd + 1) * 128], ident_r)
                    nc.vector.tensor_scalar_add(qTp[:, kd, :], ptile[:, 0:512],
                                                wcm_sb[:, kd:kd + 1])

                # ---- cN8: fp8 copy of c in natural layout (Pool engine) ----
                cN8 = sb.tile([128, NT, D], FP8, tag="cN8")
                for tq in range(4):
                    nc.gpsimd.tensor_copy(cN8[:, tq * 4:(tq + 1) * 4, :],
                                          cSt_b[:, tq * 4:(tq + 1) * 4, :])

                # ---- v = q @ wq = wq.qTp - K; ev = exp(v) row + col forms ----
                pv = ps.tile([128, 1024], F32, tag="mw")
                for kd in range(NK):
                    nc.tensor.matmul(pv[0:1, 0:512], wq_sb[:, kd:kd + 1],
                                     qTp[:, kd, :],
                                     start=(kd == 0), stop=(kd == NK - 1))
                ev_row = sb.tile([1, Lq], F32, tag="evrow")
                nc.scalar.activation(out=ev_row, in_=pv[0:1, 0:512], func=AF.Exp,
                                     bias=negK[0:1, 0:1])
                nc.sync.dma_start(out=ev_dram[:], in_=ev_row[0:1, :])
                ev_colf = sb.tile([128, NG], F32, tag="evcolf")
                nc.sync.dma_start(
                    out=ev_colf, in_=ev_dram.rearrange("(g p) -> p g", p=128))
                ev_col8 = sb.tile([128, NG, 1], FP8, tag="evcol8")
                nc.vector.tensor_copy(ev_col8[:, :, 0], ev_colf)

                # ---- chunked c transposes + ST = q~T.T @ cT -> E1T8 ----
                # cT lives as a rotating [128, NK, 512] chunk; per chunk,
                # transpose 4 context row-blocks then score all 4 g-blocks.
                cTb = sb.tile([128, NK, Lc], BF16, tag="cTb")
                E1T8 = sb.tile([128, NG, Lc], FP8, tag="E8")
                s1p = sb.tile([128, NG, 4], F32, tag="s1p")
                s1s = sb.tile([128, NG], F32, tag="s1s")
                invs1 = sb.tile([128, NG], F32, tag="invs1")
                invs1q = sb.tile([128, NG], F32, tag="invs1q")
                qn8 = sb.tile([128, NG, D], FP8, tag="qn8")
                for ic in range(4):
                    cTc = sb.tile([128, NK, 512], F32R, tag="cTc", bufs=2)
                    sl = slice(ic * 512, (ic + 1) * 512)
                    for kd in range(NK):
                        ptile = pt.tile([128, 512], F32R, tag="tr")
                        for t4 in range(4):
                            t = ic * 4 + t4
                            nc.tensor.transpose(
                                ptile[:, t4 * 128:(t4 + 1) * 128],
                                cSt_b[:, t, kd * 128:(kd + 1) * 128], ident_r)
                        nc.vector.tensor_scalar_mul(cTc[:, kd, :], ptile,
                                                    wm_sb[:, kd:kd + 1])
                        nc.scalar.activation(out=cTb[:, kd, sl], in_=ptile, func=AF.Copy)
                    for g in range(NG):
                        pm = ps.tile([128, 512], F32, tag="mw")
                        for kd in range(NK):
                            nc.tensor.matmul(
                                pm, qTp[:, kd, g * 128:(g + 1) * 128],
                                cTc[:, kd, :],
                                start=(kd == 0), stop=(kd == NK - 1))
                        nc.scalar.activation(
                            out=E1T8[:, g, sl], in_=pm,
                            func=AF.Exp, accum_out=s1p[:, g, ic:ic + 1])
                for g in range(NG):
                    nc.vector.reduce_sum(out=s1s[:, g:g + 1], in_=s1p[:, g, :], axis=AX.X)
                    nc.vector.reciprocal(out=invs1[:, g:g + 1], in_=s1s[:, g:g + 1])
                    nc.vector.tensor_scalar_mul(invs1q[:, g:g + 1], invs1[:, g:g + 1], QS)
                    # qn8 = (q/s1)*QS in fp8, from f32r staging
                    nc.vector.tensor_scalar_mul(qn8[:, g, :], qSt_b[:, g, :],
                                                invs1q[:, g:g + 1])

                # ---- s2[i] = sum_j ev[j] * E1T[j,i]  (fp8, plain) ----
                s2row = sb.tile([1, Lc], BF16, tag="s2row")
                for ic2 in range(2):
                    ps2 = ps.tile([128, 1024], F32, tag="mw")
                    for half in range(2):
                        ic = ic2 * 2 + half
                        for g in range(NG):
                            nc.tensor.matmul(
                                ps2[0:1, half * 512:(half + 1) * 512],
                                ev_col8[:, g, 0:1],
                                E1T8[:, g, ic * 512:(ic + 1) * 512],
                                start=(g == 0), stop=(g == NG - 1))
                    nc.scalar.activation(out=s2row[0:1, ic2 * 1024:(ic2 + 1) * 1024],
                                         in_=ps2[0:1, :], func=AF.Copy)
                nc.sync.dma_start(out=s2_dram[:], in_=s2row[0:1, :])
                s2c = sb.tile([128, NT], BF16, tag="s2c")
                nc.sync.dma_start(out=s2c, in_=s2_dram.rearrange("(t p) -> p t", p=128))
                invs2c = sb.tile([128, NT], F32, tag="invs2c")
                nc.vector.reciprocal(out=invs2c, in_=s2c)

                # ---- AT = (q/s1).T @ E1T fp8-DR; cAT = cTb*AT ----
                # (issued before the G transposes so the s2 DMA roundtrip
                #  hides behind PE work)
                AT = sb.tile([128, NK, Lc], BF16, tag="AT")
                cAT = sb.tile([128, NK, Lc], BF16, tag="cAT")
                for ic2 in range(2):
                    for kd in range(NK):
                        pm = ps.tile([128, 1024], F32, tag="mw")
                        for half in range(2):
                            ic = ic2 * 2 + half
                            for g2 in range(NG // 2):
                                nc.tensor.matmul(
                                    pm[:, half * 512:(half + 1) * 512],
                                    qn8[:, g2 * 2:g2 * 2 + 2, kd * 128:(kd + 1) * 128],
                                    E1T8[:, g2 * 2:g2 * 2 + 2, ic * 512:(ic + 1) * 512],
                                    start=(g2 == 0), stop=(g2 == NG // 2 - 1),
                                    perf_mode=DR)
                        sl = slice(ic2 * 1024, (ic2 + 1) * 1024)
                        nc.scalar.activation(out=AT[:, kd, sl], in_=pm,
                                             func=AF.Copy, scale=1.0 / QS)
                        nc.vector.tensor_mul(cAT[:, kd, sl], cTb[:, kd, sl], AT[:, kd, sl])

                # ---- G[i,j] = E1T[j,i] / s2[i]  (fp8 PE transposes) ----
                # (fp8 PE transpose writes with element step 2; read strided)
                G8 = sb.tile([128, NT, Lq], FP8, tag="G8")
                for t in range(NT):
                    ptb = pt.tile([128, 1024], FP8, tag="tr")
                    for g in range(NG):
                        nc.tensor.transpose(
                            ptb[:, g * 256:(g + 1) * 256:2],
                            E1T8[:, g, t * 128:(t + 1) * 128], ident8)
                    nc.vector.tensor_scalar_mul(G8[:, t, :], ptb[:, 0:1024:2],
                                                invs2c[:, t:t + 1])

                # ---- Y = diag(ev/s1*YS) (G.T @ c)  (fp8 DR) ----
                ysc = sb.tile([128, NG], F32, tag="ysc")
                nc.vector.tensor_mul(ysc, ev_colf, invs1)
                ysc_s = sb.tile([128, NG], F32, tag="yscs")
                nc.vector.tensor_scalar_mul(ysc_s, ysc, YS)
                Y8 = sb.tile([128, NG, D], FP8, tag="Y8")
                for g2 in range(2):
                    pm = ps.tile([128, 1024], F32, tag="mw")
                    # interleave the two PSUM halves per t2 step: consecutive
                    # accumulates into the same PSUM region stall the PE
                    for t2 in range(NT // 2):
                        for half in range(2):
                            g = g2 * 2 + half
                            nc.tensor.matmul(
                                pm[:, half * 512:(half + 1) * 512],
                                G8[:, t2 * 2:t2 * 2 + 2, g * 128:(g + 1) * 128],
                                cN8[:, t2 * 2:t2 * 2 + 2, :],
                                start=(t2 == 0), stop=(t2 == NT // 2 - 1),
                                perf_mode=DR)
                    for half in range(2):
                        g = g2 * 2 + half
                        nc.vector.tensor_scalar_mul(
                            Y8[:, g, :], pm[:, half * 512:(half + 1) * 512],
                            ysc_s[:, g:g + 1])

                # ---- BmT = Y.T @ E1T (fp8 DR); times cTb, descale 1/YS ----
                BmT = sb.tile([128, NK, Lc], BF16, tag="BmT")
                for ic2 in range(2):
                    for kd in range(NK):
                        pm = ps.tile([128, 1024], F32, tag="mw")
                        for half in range(2):
                            ic = ic2 * 2 + half
                            for g2 in range(NG // 2):
                                nc.tensor.matmul(
                                    pm[:, half * 512:(half + 1) * 512],
                                    Y8[:, g2 * 2:g2 * 2 + 2, kd * 128:(kd + 1) * 128],
                                    E1T8[:, g2 * 2:g2 * 2 + 2, ic * 512:(ic + 1) * 512],
                                    start=(g2 == 0), stop=(g2 == NG // 2 - 1),
                                    perf_mode=DR)
                        sl = slice(ic2 * 1024, (ic2 + 1) * 1024)
                        bt = sb.tile([128, 1024], BF16, tag="bmtmp", bufs=2)
                        nc.scalar.activation(out=bt, in_=pm,
                                             func=AF.Copy, scale=1.0 / YS)
                        nc.vector.tensor_mul(BmT[:, kd, sl], bt, cTb[:, kd, sl])

                # ---- prefetch next batch's inputs (WAR deps via tags) ----
                if b + 1 < BPC:
                    next_stage = load_stage(b + 1)

                # ---- out = c@W1 + A@W2 + cA@W3 + cB@W4 + br ----
                for t2 in range(NT // 2):
                    pm = ps.tile([128, 1024], F32, tag="mw")
                    for half in range(2):
                        t = t2 * 2 + half
                        first = True
                        for si, src in enumerate((cTb, AT, cAT, BmT)):
                            for kd in range(NK):
                                nc.tensor.matmul(
                                    pm[:, half * 512:(half + 1) * 512],
                                    src[:, kd, t * 128:(t + 1) * 128],
                                    W_sb[:, si * NK + kd, :], start=first, stop=False)
                                first = False
                        nc.tensor.matmul(pm[:, half * 512:(half + 1) * 512],
                                         ones1b, br_sb, start=False, stop=True)
                    ot = sb.tile([128, 2, 512], F32, tag="outst", bufs=3)
                    nc.vector.tensor_copy(ot, pm)
                    nc.sync.dma_start(
                        out=out2[b].rearrange("(u p) e -> p u e", p=128)[:, t2 * 2:t2 * 2 + 2, :],
                        in_=ot)
                return next_stage

            stage = stage0
            for b in range(BPC):
                stage = one_batch(b, stage)

    nc.compile()
    return nc


class Runner:
    """Persistent SPMD runner: jit once, execute many times."""

    def __init__(self, nc):
        import jax
        from jax.experimental.shard_map import shard_map
        from jax.sharding import Mesh, PartitionSpec

        bass2jax.install_neuronx_cc_hook()
        self.nc = nc
        self.jax = jax

        partition_name = (
            nc.partition_id_tensor.name if nc.partition_id_tensor else None
        )
        in_names, out_names, out_avals, zero_shapes = [], [], [], []
        for alloc in nc.m.functions[0].allocations:
            if not isinstance(alloc, mybir.MemoryLocationSet):
                continue
            name = alloc.memorylocations[0].name
            if alloc.kind == "ExternalInput":
                if name != partition_name:
                    in_names.append(name)
            elif alloc.kind == "ExternalOutput":
                shape = tuple(alloc.tensor_shape)
                dtype = mybir.dt.np(alloc.dtype)
                out_names.append(name)
                out_avals.append(jax.core.ShapedArray(shape, dtype))
                zero_shapes.append((shape, dtype))
        self.in_names = list(in_names)
        self.out_names = out_names
        self.out_avals = out_avals
        self.zero_shapes = zero_shapes
        n_params = len(in_names)
        n_outs = len(out_names)

        all_in_names = list(in_names) + list(out_names)
        if partition_name is not None:
            all_in_names.append(partition_name)

        def _body(*args):
            operands = list(args)
            if partition_name is not None:
                operands.append(bass2jax.partition_id_tensor())
            outs = bass2jax._bass_exec_p.bind(
                *operands,
                out_avals=tuple(out_avals),
                in_names=tuple(all_in_names),
                out_names=tuple(out_names),
                lowering_input_output_aliases=(),
                sim_require_finite=True,
                sim_require_nnan=True,
                nc=nc,
            )
            return tuple(outs)

        devices = jax.devices()[:N_CORES]
        mesh = Mesh(np.asarray(devices), ("core",))
        in_specs = (PartitionSpec("core"),) * (n_params + n_outs)
        out_specs = (PartitionSpec("core"),) * n_outs
        self.fn = jax.jit(
            shard_map(_body, mesh=mesh, in_specs=in_specs,
                      out_specs=out_specs, check_rep=False),
            keep_unused=True,
        )

    def concat_inputs(self, in_maps):
        return [
            np.concatenate([np.asarray(m[name]) for m in in_maps], axis=0)
            for name in self.in_names
        ]

    def zeros(self):
        return [
            np.zeros((N_CORES * s[0], *s[1:]), d) for (s, d) in self.zero_shapes
        ]

    def run_device(self, concat_in, zeros):
        """Execute; returns list of global (N_CORES*dim0, ...) jax arrays."""
        out = self.fn(*concat_in, *zeros)
        self.jax.block_until_ready(out)
        return out

    def run(self, in_maps):
        outs = self.run_device(self.concat_inputs(in_maps), self.zeros())
        return [
            {
                name: np.asarray(outs[i]).reshape(
                    N_CORES, *self.out_avals[i].shape)[c]
                for i, name in enumerate(self.out_names)
            }
            for c in range(N_CORES)
        ]


_CACHED = {}


def _get_runner(**kw):
    key = tuple(sorted(kw.items()))
    if key not in _CACHED:
        _CACHED[key] = Runner(build_program(**kw))
    return _CACHED[key]


def make_in_maps(context, question, w0, wr, br):
    return [
        {
            "c2": context[c * BPC:(c + 1) * BPC],
            "q2": question[c * BPC:(c + 1) * BPC],
            "w0": w0,
            "wr": wr,
            "br": br,
        }
        for c in range(N_CORES)
    ]


def kernel(context, question, w0, wr, br):
    context = np.ascontiguousarray(np.asarray(context, dtype=np.float32))
    question = np.ascontiguousarray(np.asarray(question, dtype=np.float32))
    w0 = np.ascontiguousarray(np.asarray(w0, dtype=np.float32))
    wr = np.ascontiguousarray(np.asarray(wr, dtype=np.float32))
    br = np.ascontiguousarray(np.asarray(br, dtype=np.float32))

    runner = _get_runner()
    res = runner.run(make_in_maps(context, question, w0, wr, br))
    return np.concatenate([res[c]["out2"] for c in range(N_CORES)], axis=0)


# revision 33
# speedup vs baseline: 1.2319x; 1.1186x over previous
"""ContextQueryAttention Trainium2 kernel.

Reference computation (per batch b):
    S = (c@wc)[:,None] + (q@wq)[None,:] + (c*wm) @ q.T        # (Lc, Lq)
    S1 = softmax(S, axis=0)  (over context dim i)
    S2 = softmax(S, axis=1)  (over question dim j)
    A  = S1 @ q
    Bm = (S1 @ S2.T) @ c
    out = [c, A, c*A, c*Bm] @ wr + br

Algebraic restructuring (same as the bf16 baseline):
  * Bm = S1 @ (S2.T @ c)   -- avoids the (Lc,Lc) intermediate entirely.
  * q~ = wm*q + wc (per-feature). Then ST = q~ @ c.T gives the i-softmax
    logits directly (the q@wq term is constant along i and cancels).
  * exp() without max subtraction (S is unit-scale; E = e^S <= ~240,
    which also fits fp8e4m3's range).
  * softmax normalizers fold into downstream operands, never rescaling
    the big exp(ST) matrix.

Precision plan (validated in numpy against the fp32 reference):
  * score path f32r (full-rate on PE for wide moving operands).
  * E1T, ev, G, q/s1, Y, c stored fp8e4m3; the four probability-weighted
    matmuls (s2, AT, Y, BmT) run in DoubleRow perf mode (2 k-tiles per
    instruction, 0.5 cyc/row).
  * q/s1 and Y are pre-scaled by QS/YS (pow2) before fp8 quantization to
    stay in e4m3's normal range; descale folds into PSUM->SBUF copies.
  * final (Lc,4D)@(4D,D) projection stays bf16 (fp8 fails tolerance).

Sharding: pure data parallel over batch: 16 batches -> 8 cores x 2.
"""

import numpy as np

import concourse.bass as bass
import concourse.tile as tile
from concourse import bacc, mybir
from concourse import bass2jax
from concourse.masks import make_identity

N_CORES = 8
B, Lc, Lq, D = 16, 2048, 512, 512
BPC = B // N_CORES  # batches per core

F32 = mybir.dt.float32
F32R = mybir.dt.float32r
BF16 = mybir.dt.bfloat16
FP8 = mybir.dt.float8e4

AF = mybir.ActivationFunctionType
ALU = mybir.AluOpType
AX = mybir.AxisListType
DR = mybir.MatmulPerfMode.DoubleRow

NT = Lc // 128   # 16 context row-blocks
NG = Lq // 128   # 4 question row-blocks
NK = D // 128    # 4 feature blocks

QS = 16384.0     # fp8 pre-scale for q/s1
YS = 8192.0      # fp8 pre-scale for Y
CS = 1024.0      # fp8 pre-scale for c/s2


def build_program():
    nc = bacc.Bacc(None, target_bir_lowering=False)

    # f32r tiles are bit-compatible with f32; np mapping stays float32.
    c2 = nc.declare_dram_parameter("c2", [BPC, Lc, D], F32R, isOutput=False)
    q2 = nc.declare_dram_parameter("q2", [BPC, Lq, D], F32R, isOutput=False)
    w0 = nc.declare_dram_parameter("w0", [3 * D], F32, isOutput=False)
    wr = nc.declare_dram_parameter("wr", [4 * D, D], F32, isOutput=False)
    br = nc.declare_dram_parameter("br", [D], F32, isOutput=False)
    out2 = nc.declare_dram_parameter("out2", [BPC, Lc, D], F32, isOutput=True)
    ev_dram = nc.dram_tensor("ev_scratch", [Lq], F32)
    s2_dram = nc.dram_tensor("s2_scratch", [Lc], BF16)

    with tile.TileContext(nc) as tc:
        with (
            tc.tile_pool(name="sb", bufs=1) as sb,
            tc.tile_pool(name="ps", bufs=2, space="PSUM") as ps,
            tc.tile_pool(name="pt", bufs=2, space="PSUM") as pt,
        ):
            # ---- constants ----
            ident_f = sb.tile([128, 128], F32, tag="identf")
            make_identity(nc, ident_f)
            ident_r = sb.tile([128, 128], F32R, tag="identr")
            nc.vector.tensor_copy(ident_r, ident_f)
            ident8 = sb.tile([128, 128], FP8, tag="ident8")
            nc.vector.tensor_copy(ident8, ident_f)
            ones1_f = sb.tile([1, 128], F32, tag="ones1f")
            nc.vector.memset(ones1_f, 1.0)
            ones1b = sb.tile([1, 128], BF16, tag="ones1b")
            nc.vector.tensor_copy(ones1b, ones1_f)

            wc_sb = sb.tile([128, NK], F32, tag="wc")
            wm_sb = sb.tile([128, NK], F32, tag="wm")
            wq_sb = sb.tile([128, NK], F32R, tag="wq")
            nc.sync.dma_start(out=wc_sb, in_=w0[0:D].rearrange("(k p) -> p k", p=128))
            nc.sync.dma_start(out=wm_sb, in_=w0[2 * D:3 * D].rearrange("(k p) -> p k", p=128))
            nc.gpsimd.dma_start(out=wq_sb, in_=w0[D:2 * D].rearrange("(k p) -> p k", p=128))

            # wcm = wc/wm, folded into the q transpose copy; the score matmul
            # becomes (q + wcm)^T (wm*c) and ev gets an exp-bias correction
            # K = sum_d wq_d*wcm_d (error from this rewrite is O(wc*2^-11)).
            wcm_sb = sb.tile([128, NK], F32, tag="wcm")
            nc.vector.reciprocal(out=wcm_sb, in_=wm_sb)
            nc.vector.tensor_mul(wcm_sb, wcm_sb, wc_sb)
            wqm = sb.tile([128, NK], F32, tag="wqm")
            nc.vector.tensor_mul(wqm, wq_sb, wcm_sb)
            ones_col = sb.tile([128, 1], F32, tag="onescol")
            nc.vector.memset(ones_col, 1.0)
            pk = ps.tile([1, 8], F32, tag="mw")
            nc.tensor.matmul(pk[0:1, 0:NK], ones_col, wqm, start=True, stop=True)
            negK = sb.tile([1, 1], F32, tag="negK")
            nc.vector.reduce_sum(out=negK, in_=pk[0:1, 0:NK], axis=AX.X)
            nc.vector.tensor_scalar_mul(negK, negK, -1.0)

            br_sb = sb.tile([1, D], BF16, tag="br")
            nc.gpsimd.dma_start(out=br_sb, in_=br.rearrange("(a e) -> a e", a=1))

            # ---- batch input staging loads (batch 0 first, then
            #      prefetched from inside one_batch for b+1) ----
            wr_r = wr.rearrange("(t p) e -> p t e", p=128)

            def load_stage(b):
                # q first: the PE pipeline starts with the q transposes
                qSt_b = sb.tile([128, NG, D], F32R, tag="qstage")
                nc.sync.dma_start(
                    out=qSt_b, in_=q2[b].rearrange("(g p) d -> p g d", p=128))
                c_rb = c2[b].rearrange("(t p) d -> p t d", p=128)
                cSt_b = sb.tile([128, NT, D], F32R, tag="cstage")
                for tq in range(4):
                    nc.sync.dma_start(out=cSt_b[:, tq * 4:(tq + 1) * 4, :],
                                      in_=c_rb[:, tq * 4:(tq + 1) * 4, :])
                return cSt_b, qSt_b

            stage0 = load_stage(0)

            # ---- wr: f32 HWDGE staging + Pool cast to bf16 (quarters,
            #      single buffer: few SP queue slots, done during score) ----
            W_sb = sb.tile([128, 4 * NK, D], BF16, tag="W")
            for quarter in range(4):
                Wst = sb.tile([128, 4, D], F32, tag="Wst", bufs=1)
                nc.sync.dma_start(out=Wst, in_=wr_r[:, quarter * 4:(quarter + 1) * 4, :])
                nc.gpsimd.tensor_copy(W_sb[:, quarter * 4:(quarter + 1) * 4, :], Wst)

            def one_batch(b, stage):
                cSt_b, qSt_b = stage
                next_stage = None

                # ---- q transposes; qTp = qT + wc/wm fused on the copy ----
                qTp = sb.tile([128, NK, Lq], F32R, tag="qTp")
                for kd in range(NK):
                    ptile = pt.tile([128, 1024], F32R, tag="tr")
                    for g in range(NG):
                        nc.tensor.transpose(
                            ptile[:, g * 128:(g + 1) * 128],
                            qSt_b[:, g, kd * 128:(kd + 1) * 128], ident_r)
                    nc.vector.tensor_scalar_add(qTp[:, kd, :], ptile[:, 0:512],
                                                wcm_sb[:, kd:kd + 1])

                # ---- v = q @ wq = wq.qTp - K; ev = exp(v) row + col forms ----
                pv = ps.tile([128, 1024], F32, tag="mw")
                for kd in range(NK):
                    nc.tensor.matmul(pv[0:1, 0:512], wq_sb[:, kd:kd + 1],
                                     qTp[:, kd, :],
                                     start=(kd == 0), stop=(kd == NK - 1))
                ev_row = sb.tile([1, Lq], F32, tag="evrow")
                nc.scalar.activation(out=ev_row, in_=pv[0:1, 0:512], func=AF.Exp,
                                     bias=negK[0:1, 0:1])
                nc.sync.dma_start(out=ev_dram[:], in_=ev_row[0:1, :])
                ev_colf = sb.tile([128, NG], F32, tag="evcolf")
                nc.sync.dma_start(
                    out=ev_colf, in_=ev_dram.rearrange("(g p) -> p g", p=128))
                ev_col8 = sb.tile([128, NG, 1], FP8, tag="evcol8")
                nc.vector.tensor_copy(ev_col8[:, :, 0], ev_colf)

                # ---- chunked c transposes + ST = q~T.T @ cT -> E1T8 ----
                # cT lives as a rotating [128, NK, 512] chunk; per chunk,
                # transpose 4 context row-blocks then score all 4 g-blocks.
                cTb = sb.tile([128, NK, Lc], BF16, tag="cTb")
                E1T8 = sb.tile([128, NG, Lc], FP8, tag="E8")
                s1p = sb.tile([128, NG, 4], F32, tag="s1p")
                s1s = sb.tile([128, NG], F32, tag="s1s")
                invs1 = sb.tile([128, NG], F32, tag="invs1")
                invs1q = sb.tile([128, NG], F32, tag="invs1q")
                qn8 = sb.tile([128, NG, D], FP8, tag="qn8")
                for ic in range(4):
                    cTc = sb.tile([128, NK, 512], F32R, tag="cTc", bufs=2)
                    sl = slice(ic * 512, (ic + 1) * 512)
                    for kd in range(NK):
                        ptile = pt.tile([128, 512], F32R, tag="tr")
                        for t4 in range(4):
                            t = ic * 4 + t4
                            nc.tensor.transpose(
                                ptile[:, t4 * 128:(t4 + 1) * 128],
                                cSt_b[:, t, kd * 128:(kd + 1) * 128], ident_r)
                        nc.vector.tensor_scalar_mul(cTc[:, kd, :], ptile,
                                                    wm_sb[:, kd:kd + 1])
                        nc.scalar.activation(out=cTb[:, kd, sl], in_=ptile, func=AF.Copy)
                    for g in range(NG):
                        pm = ps.tile([128, 512], F32, tag="mw")
                        for kd in range(NK):
                            nc.tensor.matmul(
                                pm, qTp[:, kd, g * 128:(g + 1) * 128],
                                cTc[:, kd, :],
                                start=(kd == 0), stop=(kd == NK - 1))
                        nc.scalar.activation(
                            out=E1T8[:, g, sl], in_=pm,
                            func=AF.Exp, accum_out=s1p[:, g, ic:ic + 1])
                for g in range(NG):
                    nc.vector.reduce_sum(out=s1s[:, g:g + 1], in_=s1p[:, g, :], axis=AX.X)
                    nc.vector.reciprocal(out=invs1[:, g:g + 1], in_=s1s[:, g:g + 1])
                    nc.vector.tensor_scalar_mul(invs1q[:, g:g + 1], invs1[:, g:g + 1], QS)
                    # qn8 = (q/s1)*QS in fp8, from f32r staging
                    nc.vector.tensor_scalar_mul(qn8[:, g, :], qSt_b[:, g, :],
                                                invs1q[:, g:g + 1])

                # ---- s2[i] = sum_j ev[j] * E1T[j,i]  (fp8, plain) ----
                s2row = sb.tile([1, Lc], BF16, tag="s2row")
                for ic2 in range(2):
                    ps2 = ps.tile([128, 1024], F32, tag="mw")
                    for half in range(2):
                        ic = ic2 * 2 + half
                        for g in range(NG):
                            nc.tensor.matmul(
                                ps2[0:1, half * 512:(half + 1) * 512],
                                ev_col8[:, g, 0:1],
                                E1T8[:, g, ic * 512:(ic + 1) * 512],
                                start=(g == 0), stop=(g == NG - 1))
                    nc.scalar.activation(out=s2row[0:1, ic2 * 1024:(ic2 + 1) * 1024],
                                         in_=ps2[0:1, :], func=AF.Copy)
                nc.sync.dma_start(out=s2_dram[:], in_=s2row[0:1, :])
                s2c = sb.tile([128, NT], BF16, tag="s2c")
                nc.sync.dma_start(out=s2c, in_=s2_dram.rearrange("(t p) -> p t", p=128))
                invs2c = sb.tile([128, NT], F32, tag="invs2c")
                nc.vector.reciprocal(out=invs2c, in_=s2c)
                invs2cs = sb.tile([128, NT], F32, tag="invs2cs")
                nc.gpsimd.tensor_scalar_mul(invs2cs, invs2c, CS)

                # ---- G = E1T transposed, plain fp8 (no s2 dependency:
                #      1/s2 is folded into the Y operand cN8s instead) ----
                # (fp8 PE transpose writes with element step 2; read strided)
                G8 = sb.tile([128, NT, Lq], FP8, tag="G8")
                for t in range(NT):
                    ptb = pt.tile([128, 1024], FP8, tag="tr")
                    for g in range(NG):
                        nc.tensor.transpose(
                            ptb[:, g * 256:(g + 1) * 256:2],
                            E1T8[:, g, t * 128:(t + 1) * 128], ident8)
                    if t % 2 == 0:
                        nc.vector.tensor_copy(G8[:, t, :], ptb[:, 0:1024:2])
                    else:
                        nc.scalar.activation(out=G8[:, t, :], in_=ptb[:, 0:1024:2],
                                             func=AF.Copy)

                # ---- AT = (q/s1).T @ E1T fp8-DR; cAT = cTb*AT ----
                AT = sb.tile([128, NK, Lc], BF16, tag="AT")
                cAT = sb.tile([128, NK, Lc], BF16, tag="cAT")
                for ic2 in range(2):
                    for kd in range(NK):
                        pm = ps.tile([128, 1024], F32, tag="mw")
                        for half in range(2):
                            ic = ic2 * 2 + half
                            for g2 in range(NG // 2):
                                nc.tensor.matmul(
                                    pm[:, half * 512:(half + 1) * 512],
                                    qn8[:, g2 * 2:g2 * 2 + 2, kd * 128:(kd + 1) * 128],
                                    E1T8[:, g2 * 2:g2 * 2 + 2, ic * 512:(ic + 1) * 512],
                                    start=(g2 == 0), stop=(g2 == NG // 2 - 1),
                                    perf_mode=DR)
                        sl = slice(ic2 * 1024, (ic2 + 1) * 1024)
                        nc.scalar.activation(out=AT[:, kd, sl], in_=pm,
                                             func=AF.Copy, scale=1.0 / QS)
                        nc.vector.tensor_mul(cAT[:, kd, sl], cTb[:, kd, sl], AT[:, kd, sl])

                # ---- cN8s = fp8(c/s2*CS), split DVE/ACT for latency ----
                cN8s = sb.tile([128, NT, D], FP8, tag="cN8")
                for t in range(NT):
                    if t % 2 == 0:
                        nc.vector.tensor_scalar_mul(cN8s[:, t, :], cSt_b[:, t, :],
                                                    invs2cs[:, t:t + 1])
                    else:
                        nc.scalar.activation(out=cN8s[:, t, :], in_=cSt_b[:, t, :],
                                             func=AF.Copy, scale=invs2cs[:, t:t + 1])

                # ---- Y = diag(ev/s1*YS/CS) (G.T @ c/s2*CS)  (fp8 DR) ----
                ysc = sb.tile([128, NG], F32, tag="ysc")
                nc.vector.tensor_mul(ysc, ev_colf, invs1)
                ysc_s = sb.tile([128, NG], F32, tag="yscs")
                nc.vector.tensor_scalar_mul(ysc_s, ysc, YS / CS)
                Y8 = sb.tile([128, NG, D], FP8, tag="Y8")
                for g2 in range(2):
                    pm = ps.tile([128, 1024], F32, tag="mw")
                    # interleave the two PSUM halves per t2 step: consecutive
                    # accumulates into the same PSUM region stall the PE
                    for t2 in range(NT // 2):
                        for half in range(2):
                            g = g2 * 2 + half
                            nc.tensor.matmul(
                                pm[:, half * 512:(half + 1) * 512],
                                G8[:, t2 * 2:t2 * 2 + 2, g * 128:(g + 1) * 128],
                                cN8s[:, t2 * 2:t2 * 2 + 2, :],
                                start=(t2 == 0), stop=(t2 == NT // 2 - 1),
                                perf_mode=DR)
                    for half in range(2):
                        g = g2 * 2 + half
                        nc.vector.tensor_scalar_mul(
                            Y8[:, g, :], pm[:, half * 512:(half + 1) * 512],
                            ysc_s[:, g:g + 1])

                # ---- BmT = Y.T @ E1T (fp8 DR); times cTb, descale 1/YS ----
                BmT = sb.tile([128, NK, Lc], BF16, tag="BmT")
                for ic2 in range(2):
                    for kd in range(NK):
                        pm = ps.tile([128, 1024], F32, tag="mw")
                        for half in range(2):
                            ic = ic2 * 2 + half
                            for g2 in range(NG // 2):
                                nc.tensor.matmul(
                                    pm[:, half * 512:(half + 1) * 512],
                                    Y8[:, g2 * 2:g2 * 2 + 2, kd * 128:(kd + 1) * 128],
                                    E1T8[:, g2 * 2:g2 * 2 + 2, ic * 512:(ic + 1) * 512],
                                    start=(g2 == 0), stop=(g2 == NG // 2 - 1),
                                    perf_mode=DR)
                        sl = slice(ic2 * 1024, (ic2 + 1) * 1024)
                        bt = sb.tile([128, 1024], BF16, tag="bmtmp", bufs=2)
                        nc.scalar.activation(out=bt, in_=pm,
                                             func=AF.Copy, scale=1.0 / YS)
                        nc.vector.tensor_mul(BmT[:, kd, sl], bt, cTb[:, kd, sl])

                # ---- prefetch next batch's inputs (WAR deps via tags) ----
                if b + 1 < BPC:
                    next_stage = load_stage(b + 1)

                # ---- out = c@W1 + A@W2 + cA@W3 + cB@W4 + br ----
                for t2 in range(NT // 2):
                    pm = ps.tile([128, 1024], F32, tag="mw")
                    for half in range(2):
                        t = t2 * 2 + half
                        first = True
                        for si, src in enumerate((cTb, AT, cAT, BmT)):
                            for kd in range(NK):
                                nc.tensor.matmul(
                                    pm[:, half * 512:(half + 1) * 512],
                                    src[:, kd, t * 128:(t + 1) * 128],
                                    W_sb[:, si * NK + kd, :], start=first, stop=False)
                                first = False
                        nc.tensor.matmul(pm[:, half * 512:(half + 1) * 512],
                                         ones1b, br_sb, start=False, stop=True)
                    ot = sb.tile([128, 2, 512], F32, tag="outst", bufs=3)
                    nc.vector.tensor_copy(ot, pm)
                    nc.sync.dma_start(
                        out=out2[b].rearrange("(u p) e -> p u e", p=128)[:, t2 * 2:t2 * 2 + 2, :],
                        in_=ot)
                return next_stage

            stage = stage0
            for b in range(BPC):
                stage = one_batch(b, stage)

    nc.compile()
    return nc


class Runner:
    """Persistent SPMD runner: jit once, execute many times."""

    def __init__(self, nc):
        import jax
        from jax.experimental.shard_map import shard_map
        from jax.sharding import Mesh, PartitionSpec

        bass2jax.install_neuronx_cc_hook()
        self.nc = nc
        self.jax = jax

        partition_name = (
            nc.partition_id_tensor.name if nc.partition_id_tensor else None
        )
        in_names, out_names, out_avals, zero_shapes = [], [], [], []
        for alloc in nc.m.functions[0].allocations:
            if not isinstance(alloc, mybir.MemoryLocationSet):
                continue
            name = alloc.memorylocations[0].name
            if alloc.kind == "ExternalInput":
                if name != partition_name:
                    in_names.append(name)
            elif alloc.kind == "ExternalOutput":
                shape = tuple(alloc.tensor_shape)
                dtype = mybir.dt.np(alloc.dtype)
                out_names.append(name)
                out_avals.append(jax.core.ShapedArray(shape, dtype))
                zero_shapes.append((shape, dtype))
        self.in_names = list(in_names)
        self.out_names = out_names
        self.out_avals = out_avals
        self.zero_shapes = zero_shapes
        n_params = len(in_names)
        n_outs = len(out_names)

        all_in_names = list(in_names) + list(out_names)
        if partition_name is not None:
            all_in_names.append(partition_name)

        def _body(*args):
            operands = list(args)
            if partition_name is not None:
                operands.append(bass2jax.partition_id_tensor())
            outs = bass2jax._bass_exec_p.bind(
                *operands,
                out_avals=tuple(out_avals),
                in_names=tuple(all_in_names),
                out_names=tuple(out_names),
                lowering_input_output_aliases=(),
                sim_require_finite=True,
                sim_require_nnan=True,
                nc=nc,
            )
            return tuple(outs)

        devices = jax.devices()[:N_CORES]
        mesh = Mesh(np.asarray(devices), ("core",))
        in_specs = (PartitionSpec("core"),) * (n_params + n_outs)
        out_specs = (PartitionSpec("core"),) * n_outs
        self.fn = jax.jit(
            shard_map(_body, mesh=mesh, in_specs=in_specs,
                      out_specs=out_specs, check_rep=False),
            keep_unused=True,
        )

    def concat_inputs(self, in_maps):
        return [
            np.concatenate([np.asarray(m[name]) for m in in_maps], axis=0)
            for name in self.in_names
        ]

    def zeros(self):
        return [
            np.zeros((N_CORES * s[0], *s[1:]), d) for (s, d) in self.zero_shapes
        ]

    def run_device(self, concat_in, zeros):
        """Execute; returns list of global (N_CORES*dim0, ...) jax arrays."""
        out = self.fn(*concat_in, *zeros)
        self.jax.block_until_ready(out)
        return out

    def run(self, in_maps):
        outs = self.run_device(self.concat_inputs(in_maps), self.zeros())
        return [
            {
                name: np.asarray(outs[i]).reshape(
                    N_CORES, *self.out_avals[i].shape)[c]
                for i, name in enumerate(self.out_names)
            }
            for c in range(N_CORES)
        ]


_CACHED = {}


def _get_runner(**kw):
    key = tuple(sorted(kw.items()))
    if key not in _CACHED:
        _CACHED[key] = Runner(build_program(**kw))
    return _CACHED[key]


def make_in_maps(context, question, w0, wr, br):
    return [
        {
            "c2": context[c * BPC:(c + 1) * BPC],
            "q2": question[c * BPC:(c + 1) * BPC],
            "w0": w0,
            "wr": wr,
            "br": br,
        }
        for c in range(N_CORES)
    ]


def kernel(context, question, w0, wr, br):
    context = np.ascontiguousarray(np.asarray(context, dtype=np.float32))
    question = np.ascontiguousarray(np.asarray(question, dtype=np.float32))
    w0 = np.ascontiguousarray(np.asarray(w0, dtype=np.float32))
    wr = np.ascontiguousarray(np.asarray(wr, dtype=np.float32))
    br = np.ascontiguousarray(np.asarray(br, dtype=np.float32))

    runner = _get_runner()
    res = runner.run(make_in_maps(context, question, w0, wr, br))
    return np.concatenate([res[c]["out2"] for c in range(N_CORES)], axis=0)
